# revision 43
# baseline (speedup 1.0000x reference)
"""AgentAttention Trainium2 kernel (v2).

Data-parallel over batch: 32 samples -> 8 cores x 4 samples.
Device layout is channels-major: activations live as (c, t), bf16.

Per-sample pipeline:
  qk^T  = Wqk^T.T @ xs^T              (bf16 matmuls, f32 PSUM)
  v_t   = xs^T.T @ Wv^T               (tokens-major bf16 + ones col for A1V)
  v^T   = Wv^T.T @ xs^T               (fp8, zero-padded 34x34 image for dwc)
  agents^T: strided-window sums of q^T on DVE (adaptive pool), scaled
  S1^T[t,(h,a)] = k^T.T @ blockdiag(agents) -> exp ACT -> *expB1 (DVE)
  A1V: agent_v + denominators via ones column; normalize -> BDagv
  S2[(h,a),t]  = blockdiag(agents).T @ q^T  -> exp ACT -> *expB2 (DVE)
  A2V: lhsT=[ones2|BDagv] -> dens at PSUM rows 0:2, data rows 2:66;
       recip -> SBUF->SBUF partition-broadcast DMA -> cross-partition mult
  dwc: fp8 DoubleRow diagonal matmuls over shifted views of padded v^T
  proj: Wp^T.T @ pre_proj -> bf16 out
Host adds proj/dwc biases and restores (b, n+1, c) order.
"""

import numpy as np
import ml_dtypes

DEBUG = False
STAGE = 99  # truncate pipeline for perf bisect
REPEAT = 0  # >0: wrap sample loop in a hardware For_i for timing
NOBCAST = False

import bass_rust
import concourse.bacc as bacc
import concourse.tile as tile
import concourse.mybir as mybir
from concourse import bass_utils


def _sv(base_ap, extra_off, dims):
    """Arbitrary-strided free-dim view: keep partition dim, replace free dims
    with explicit (stride, count) pairs, shift the element offset."""
    v = base_ap.copy()
    part = tuple(list(v.ap)[0])
    v.ap = bass_rust.VecI64Pair([part] + [tuple(d) for d in dims])
    v.offset = v.offset + extra_off
    return v

N_CORES = 8
B = 32
SPB = B // N_CORES
C = 256
NT = 1024
WIN = 32
HEADS = 8
HD = 32
AGENT = 49
POOL = 7
SCALE = HD ** -0.5

F32 = mybir.dt.float32
BF16 = mybir.dt.bfloat16
FP8 = mybir.dt.float8e4
AF = mybir.ActivationFunctionType
ALU = mybir.AluOpType
AX = mybir.AxisListType
PM = mybir.MatmulPerfMode

W_SCALE = 32.0   # host scales dwc weights by this for fp8 range
V_SCALE = 4.0    # device scales v image by this for fp8 range
W8S = 32.0       # host scales qkv weights by this for fp8 range

BINS_START = [(i * WIN) // POOL for i in range(POOL)]
BINS_END = [-((-(i + 1) * WIN) // POOL) for i in range(POOL)]

# dwc taps by flat offset in the 34-wide padded image: tap (dr,dc) -> 34*dr+dc
# DoubleRow pairs must have a constant offset delta expressible as one AP dim.
DWC_PAIRS = [((0, 0), (0, 1)), ((0, 2), (1, 0)), ((1, 1), (1, 2)),
             ((2, 0), (2, 1))]
DWC_SINGLE = (2, 2)


# ----------------------------------------------------------------- host prep
def _resize_bilinear_7_to_32(b):
    src, dst = 7, 32
    coords = (np.arange(dst) + 0.5) * (src / dst) - 0.5
    i0 = np.floor(coords).astype(np.int64)
    frac = coords - i0
    i0c = np.clip(i0, 0, src - 1)
    i1c = np.clip(i0 + 1, 0, src - 1)

    def along(x, axis):
        a0 = np.take(x, i0c, axis=axis)
        a1 = np.take(x, i1c, axis=axis)
        sh = [1] * x.ndim
        sh[axis] = dst
        f = frac.reshape(sh)
        return a0 * (1.0 - f) + a1 * f

    return along(along(b, -2), -1)


def _host_consts(qkv_w, proj_w, proj_b, dwc_w, dwc_b,
                 an_bias, ah_bias, aw_bias, na_bias, ha_bias, wa_bias):
    c = {}
    c["wqk"] = np.ascontiguousarray(
        qkv_w[:2 * C].T.reshape(2, 128, 2 * C) * W8S).astype(ml_dtypes.float8_e4m3)
    c["wv"] = np.ascontiguousarray(
        qkv_w[2 * C:].T.reshape(2, 128, C) * W8S).astype(ml_dtypes.float8_e4m3)
    c["wp"] = np.ascontiguousarray(
        proj_w.T.reshape(2, 128, C)).astype(ml_dtypes.bfloat16)

    # stage-1 bias, exp'ed, layout (t, 256*g + 64*h'' + a)
    pb1 = _resize_bilinear_7_to_32(an_bias).reshape(HEADS, AGENT, NT)
    pb2 = (ah_bias + aw_bias).reshape(HEADS, AGENT, NT)
    b1 = pb1 + pb2
    eb1 = np.zeros((NT, 512), np.float32)
    for g in range(2):
        for hh in range(4):
            eb1[:, 256 * g + 64 * hh:256 * g + 64 * hh + AGENT] = \
                b1[4 * g + hh].T
    c["expB1"] = np.exp(eb1).reshape(NT // 128, 128, 512).astype(ml_dtypes.bfloat16)

    # stage-2 bias, exp'ed, layout [pair][64*e + a, t]
    ab1 = _resize_bilinear_7_to_32(na_bias).reshape(HEADS, AGENT, NT)
    ha = ha_bias.reshape(HEADS, AGENT, WIN)
    wa = wa_bias.reshape(HEADS, AGENT, WIN)
    b2 = (ab1.reshape(HEADS, AGENT, WIN, WIN)
          + ha[:, :, :, None] + wa[:, :, None, :]).reshape(HEADS, AGENT, NT)
    eb2 = np.zeros((4, 113, NT), np.float32)
    for p in range(4):
        for e in range(2):
            eb2[p, 64 * e:64 * e + AGENT] = np.exp(b2[2 * p + e])
    c["expB2"] = eb2.astype(ml_dtypes.bfloat16)

    # pooled-agent scale (fold pool mean + attention scale)
    sz = np.array([BINS_END[i] - BINS_START[i] for i in range(POOL)], np.float32)
    sa = SCALE / (sz[:, None] * sz[None, :])
    c["sa"] = np.broadcast_to(sa.reshape(1, AGENT), (128, AGENT)).astype(np.float32).copy()

    # dwc diagonal blocks, fp8, paired for DoubleRow, scaled by W_SCALE
    w3 = dwc_w.reshape(C, 3, 3).astype(np.float32) * W_SCALE
    w3p = np.zeros((2, 4, 128, 2, 128), np.float32)
    for ci in range(2):
        for pr, (t0, t1) in enumerate(DWC_PAIRS):
            for kt, (dr, dc) in enumerate((t0, t1)):
                np.fill_diagonal(w3p[ci, pr, :, kt, :],
                                 w3[128 * ci:128 * ci + 128, dr, dc])
    c["w3p"] = w3p.astype(ml_dtypes.float8_e4m3)
    w3s = np.zeros((2, 128, 128), np.float32)
    for ci in range(2):
        np.fill_diagonal(w3s[ci], w3[128 * ci:128 * ci + 128,
                                     DWC_SINGLE[0], DWC_SINGLE[1]])
    c["w3s"] = w3s.astype(ml_dtypes.float8_e4m3)

    # ones: col0 -> rows 0:49 (e0 den), col1 -> rows 64:113 (e1 den)
    o2 = np.zeros((113, 2), np.float32)
    o2[0:AGENT, 0] = 1.0
    o2[64:64 + AGENT, 1] = 1.0
    c["ones2"] = o2.astype(ml_dtypes.bfloat16)
    c["onev"] = np.ones((128, 1), np.float32).astype(ml_dtypes.float8_e4m3)

    # host-side output biases
    c["bias_cls"] = proj_b.astype(np.float32)
    c["bias_sp"] = (proj_b + proj_w @ dwc_b).astype(np.float32)
    return c


def _mm512(nc, out, lhsT, rhs, start, stop, n):
    for n0 in range(0, n, 512):
        n1 = min(n0 + 512, n)
        nc.tensor.matmul(out[:, n0:n1], lhsT, rhs[:, n0:n1],
                         start=start, stop=stop)


# ------------------------------------------------------------- device build
def build_nc():
    nc = bacc.Bacc("TRN2", target_bir_lowering=False, debug=False,
                   num_devices=N_CORES)
    dr = {}
    dr["xT"] = nc.dram_tensor("xT", (2, 128, SPB, NT), FP8,
                              kind="ExternalInput").ap()
    dr["xcls"] = nc.dram_tensor("xcls", (2, 128, SPB), BF16,
                                kind="ExternalInput").ap()
    dr["wqk"] = nc.dram_tensor("wqk", (2, 128, 512), FP8, kind="ExternalInput").ap()
    dr["wv"] = nc.dram_tensor("wv", (2, 128, 256), FP8, kind="ExternalInput").ap()
    dr["wp"] = nc.dram_tensor("wp", (2, 128, 256), BF16, kind="ExternalInput").ap()
    dr["expB1"] = nc.dram_tensor("expB1", (8, 128, 512), BF16, kind="ExternalInput").ap()
    dr["expB2"] = nc.dram_tensor("expB2", (4, 113, NT), BF16, kind="ExternalInput").ap()
    dr["sa"] = nc.dram_tensor("sa", (128, AGENT), F32, kind="ExternalInput").ap()
    dr["w3p"] = nc.dram_tensor("w3p", (2, 4, 128, 2, 128), FP8, kind="ExternalInput").ap()
    dr["w3s"] = nc.dram_tensor("w3s", (2, 128, 128), FP8, kind="ExternalInput").ap()
    dr["ones2"] = nc.dram_tensor("ones2", (113, 2), BF16, kind="ExternalInput").ap()
    dr["onev"] = nc.dram_tensor("onev", (128, 1), FP8, kind="ExternalInput").ap()
    dr["y"] = nc.dram_tensor("y", (2, 128, SPB, NT + 1), BF16,
                             kind="ExternalOutput").ap()

    with tile.TileContext(nc) as tc:
        _emit(tc, dr)
    nc.compile()
    return nc


def _emit(tc, dr):
    nc = tc.nc
    from contextlib import ExitStack
    with ExitStack() as ctx:
        cpool = ctx.enter_context(tc.tile_pool(name="consts", bufs=1))
        spx = ctx.enter_context(tc.tile_pool(name="spx", bufs=2))
        sp2 = ctx.enter_context(tc.tile_pool(name="sp2", bufs=3))
        sps = ctx.enter_context(tc.tile_pool(name="sps", bufs=3))
        ps_big = ctx.enter_context(
            tc.tile_pool(name="ps_big", bufs=3, space="PSUM"))
        ps_a = ctx.enter_context(tc.tile_pool(name="ps_a", bufs=2, space="PSUM"))

        # ---- constants to SBUF
        wqk = cpool.tile([128, 2, 512], FP8)
        wv = cpool.tile([128, 2, 256], FP8)
        wp = cpool.tile([128, 2, 256], BF16)
        xcls = cpool.tile([128, 2, SPB], BF16)
        nc.sync.dma_start(xcls[:, 0, :], dr["xcls"][0])
        nc.sync.dma_start(xcls[:, 1, :], dr["xcls"][1])
        eB1 = cpool.tile([128, 8, 512], BF16)
        eB2 = cpool.tile([113, 4, NT], BF16)
        sa = cpool.tile([128, AGENT], F32)
        w3p = cpool.tile([128, 2, 4, 2, 128], FP8)
        w3s = cpool.tile([128, 2, 128], FP8)
        ones2 = cpool.tile([113, 2], BF16)
        onev = cpool.tile([128, 1], FP8)
        for ki in range(2):
            nc.sync.dma_start(wqk[:, ki, :], dr["wqk"][ki])
            nc.sync.dma_start(wv[:, ki, :], dr["wv"][ki])
            nc.sync.dma_start(wp[:, ki, :], dr["wp"][ki])
        for ti in range(8):
            nc.sync.dma_start(eB1[:, ti, :], dr["expB1"][ti])
        for p in range(4):
            nc.sync.dma_start(eB2[:, p, :], dr["expB2"][p])
        nc.sync.dma_start(sa[:], dr["sa"][:])
        for ci in range(2):
            for pr in range(4):
                nc.sync.dma_start(w3p[:, ci, pr, :, :], dr["w3p"][ci, pr])
            nc.sync.dma_start(w3s[:, ci, :], dr["w3s"][ci])
        nc.sync.dma_start(ones2[:], dr["ones2"][:])
        nc.sync.dma_start(onev[:], dr["onev"][:])

        # persistent per-sample ring tiles whose static parts are set up once
        vTps, v_ts, BD1s, BD2s, BDagvs = [], [], [], [], []
        for bi in range(3):
            vTp = cpool.tile([128, 2, 34, 34], FP8, tag=f"vTp{bi}")
            v_t = cpool.tile([128, 8, 4, 65], FP8, tag=f"v_t{bi}")
            BD1 = cpool.tile([128, 2, 256], BF16, tag=f"BD1{bi}")
            BD2 = cpool.tile([128, 4, 113], BF16, tag=f"BD2{bi}")
            BDagv = cpool.tile([113, 4, 66], BF16, tag=f"BDagv{bi}")
            nc.vector.memset(vTp[:], 0.0)
            nc.vector.memset(BD1[:], 0.0)
            nc.vector.memset(BD2[:], 0.0)
            nc.vector.memset(BDagv[:], 0.0)
            for ti in range(8):
                for p4 in range(4):
                    nc.gpsimd.tensor_copy(v_t[:, ti, p4, 64:65], onev[:])
            for p4 in range(4):
                nc.gpsimd.tensor_copy(BDagv[:, p4, 64:66], ones2[:])
            vTps.append(vTp); v_ts.append(v_t)
            BD1s.append(BD1); BD2s.append(BD2); BDagvs.append(BDagv)

        def body():
            xT = spx.tile([128, 2, SPB, NT], FP8, tag="xT")
            for ki in range(2):
                nc.sync.dma_start(xT[:, ki, :, :], dr["xT"][ki])
            outT = spx.tile([128, 2, SPB, NT + 1], BF16, tag="outT")
            for s in range(SPB):
                _sample(tc, dr, s, xT, outT, xcls, wqk, wv, wp, eB1, eB2, sa,
                        w3p, w3s, vTps[s % 3], v_ts[s % 3], BD1s[s % 3],
                        BD2s[s % 3], BDagvs[s % 3],
                        sp2, sps, ps_big, ps_a)
            for mi in range(2):
                nc.sync.dma_start(dr["y"][mi], outT[:, mi, :, :])

        if REPEAT > 0:
            with tc.For_i(0, REPEAT, 1):
                body()
        else:
            body()


def _sample(tc, dr, s, xT, outT, xcls, wqk, wv, wp, eB1, eB2, sa,
            w3p, w3s, vTp, v_t, BD1, BD2, BDagv,
            sp2, sps, ps_big, ps_a):
    nc = tc.nc
    xs = xT[:, :, s, :]  # (128, ki, 1024) spatial only (fp8)

    # ---- qk^T: 4 m-chunks, fp8 DoubleRow over the 2 k-chunks
    qk = sp2.tile([128, 4, NT], BF16, tag="qk")
    for mi in range(4):
        acc = ps_big.tile([128, NT], F32, tag="big")
        for n0 in range(0, NT, 512):
            nc.tensor.matmul(acc[:, n0:n0 + 512],
                             wqk[:, :, 128 * mi:128 * mi + 128],
                             xs[:, :, n0:n0 + 512],
                             start=True, stop=True, perf_mode=PM.DoubleRow)
        if mi >= 2:
            nc.scalar.activation(qk[:, mi, :], acc[:], AF.Copy,
                                 scale=1.0 / W8S)
        else:
            nc.vector.tensor_scalar(out=qk[:, mi, :], in0=acc[:],
                                    scalar1=1.0 / W8S, scalar2=None,
                                    op0=ALU.mult)

    # ---- v tokens-major bf16 (ones col persists at col 64), fp8 DR
    for grp in range(2):
        acc = ps_big.tile([128, 4, 256], F32, tag="big")
        for tj in range(4):
            ti = 4 * grp + tj
            nc.tensor.matmul(acc[:, tj, :],
                             xs[:, :, 128 * ti:128 * ti + 128],
                             wv[:], start=True, stop=True,
                             perf_mode=PM.DoubleRow)
        dst = v_t[:, 4 * grp:4 * grp + 4, :, 0:64]
        srcv = acc[:].rearrange("p t (a b) -> p t a b", a=4)
        if grp == 0:
            nc.vector.tensor_scalar(out=dst, in0=srcv, scalar1=1.0 / W8S,
                                    scalar2=None, op0=ALU.mult)
        else:
            nc.scalar.activation(dst, srcv, AF.Copy, scale=1.0 / W8S)

    # ---- v^T into zero-padded (34,34) fp8 image, scaled, fp8 DR
    for ci in range(2):
        acc = ps_big.tile([128, NT], F32, tag="big")
        for n0 in range(0, NT, 512):
            nc.tensor.matmul(acc[:, n0:n0 + 512],
                             wv[:, :, 128 * ci:128 * ci + 128],
                             xs[:, :, n0:n0 + 512],
                             start=True, stop=True, perf_mode=PM.DoubleRow)
        nc.scalar.activation(
            vTp[:, ci, 1:33, 1:33],
            acc[:].rearrange("p (h w) -> p h w", h=32),
            AF.Copy, scale=V_SCALE / W8S)

    if STAGE < 2:
        return
    # ---- adaptive pool of q -> agents (AG), scaled
    RP = sps.tile([128, 2, POOL, WIN], BF16, tag="RP")
    AGf = sps.tile([128, 2, AGENT], BF16, tag="AGf")
    AG = sps.tile([128, 2, AGENT], BF16, tag="AG")
    qv = qk[:, 0:2, :].rearrange("p c (h w) -> p c w h", h=WIN)
    with nc.allow_low_precision(reason="bf16 pool sums, 21-elem windows"):
        for i in range(POOL):
            nc.vector.reduce_sum(RP[:, :, i, :],
                                 qv[:, :, :, BINS_START[i]:BINS_END[i]], axis=AX.X)
        agv = AGf[:].rearrange("p c (i j) -> p c j i", j=POOL)
        for j in range(POOL):
            nc.vector.reduce_sum(agv[:, :, j, :],
                                 RP[:, :, :, BINS_START[j]:BINS_END[j]], axis=AX.X)
    nc.vector.tensor_tensor(AG[:], AGf[:],
                            sa[:].unsqueeze(1).broadcast_to((128, 2, AGENT)),
                            op=ALU.mult)

    # ---- block-diagonal agent tiles
    for g in range(2):
        for hh in range(4):
            nc.gpsimd.tensor_copy(
                BD1[32 * hh:32 * hh + 32, g, 64 * hh:64 * hh + AGENT],
                AG[32 * hh:32 * hh + 32, g, :])
    for p in range(4):
        b = 64 * (p % 2)
        for e in range(2):
            nc.gpsimd.tensor_copy(
                BD2[b + 32 * e:b + 32 * e + 32, p, 64 * e:64 * e + AGENT],
                AG[b + 32 * e:b + 32 * e + 32, p // 2, :])

    # ---- stage 1 scores^T (t, (g,h,a)) + exp + bias factor
    expS1 = sp2.tile([128, 8, 512], FP8, tag="expS1")
    for ti in range(8):
        acc = ps_a.tile([128, 512], F32, tag="a")
        for g in range(2):
            nc.tensor.matmul(acc[:, 256 * g:256 * g + 256],
                             qk[:, 2 + g, 128 * ti:128 * ti + 128],
                             BD1[:, g, :], start=True, stop=True)
        ev = expS1[:, ti, :].rearrange("p (h c) -> p h c", c=64)[:, :, 0:49]
        av = acc[:].rearrange("p (h c) -> p h c", c=64)[:, :, 0:49]
        nc.scalar.activation(ev, av, AF.Exp)
    # ---- stage 2 scores ((e,a), t) + exp + bias factor
    expS2 = sp2.tile([113, 4, NT], BF16, tag="expS2")
    for p in range(4):
        b = 64 * (p % 2)
        acc = ps_big.tile([113, NT], F32, tag="big")
        _mm512(nc, acc, BD2[b:b + 64, p, :],
               qk[b:b + 64, p // 2, :], True, True, NT)
        nc.scalar.activation(expS2[:, p, :], acc[:], AF.Exp)
    for hf in range(2):
        evh = expS1[:, 4 * hf:4 * hf + 4, :].rearrange(
            "p t (h c) -> p t h c", c=64)[:, :, :, 0:49]
        ebh = eB1[:, 4 * hf:4 * hf + 4, :].rearrange(
            "p t (h c) -> p t h c", c=64)[:, :, :, 0:49]
        nc.vector.tensor_tensor(evh, evh, ebh, op=ALU.mult)

    if STAGE < 3:
        return
    # ---- A1V: agent_v + denominators via ones column -> BDagv cols 2:66
    rec = sps.tile([128, 4, 1], F32, tag="rec")
    for p in range(4):
        acc = ps_a.tile([113, 65], F32, tag="a")
        c0 = 256 * (p // 2) + 128 * (p % 2)
        for tp in range(4):
            nc.tensor.matmul(acc[:],
                             expS1[:, 2 * tp:2 * tp + 2, c0:c0 + 113],
                             v_t[:, 2 * tp:2 * tp + 2, p, :],
                             start=(tp == 0), stop=(tp == 3),
                             perf_mode=PM.DoubleRow)
        nc.vector.reciprocal(rec[0:113, p, :], acc[:, 64:65])
        for e in range(2):
            nc.vector.tensor_scalar(
                out=BDagv[64 * e:64 * e + 49, p, 32 * e:32 * e + 32],
                in0=acc[64 * e:64 * e + 49, 32 * e:32 * e + 32],
                scalar1=rec[64 * e:64 * e + 49, p, :],
                scalar2=None, op0=ALU.mult)

    if STAGE < 4:
        return
    for hf in range(2):
        eng = nc.vector if hf == 0 else nc.gpsimd
        eng.tensor_tensor(expS2[:, 2 * hf:2 * hf + 2, :],
                          expS2[:, 2 * hf:2 * hf + 2, :],
                          eB2[:, 2 * hf:2 * hf + 2, :], op=ALU.mult)

    # ---- A2V: data rows 0:64, dens rows 64:66; bcast recip; normalize
    pre = sp2.tile([128, 2, NT + 1], BF16, tag="pre")
    for ci in range(2):
        nc.gpsimd.tensor_copy(pre[:, ci, NT:NT + 1], xcls[:, ci, s:s + 1])
    for p in range(4):
        b = 64 * (p % 2)
        ci = p // 2
        acc = ps_big.tile([128, NT], F32, tag="big")
        _mm512(nc, acc[0:66, :], BDagv[:, p, :],
               expS2[:, p, :], True, True, NT)
        rcb = sps.tile([2, NT], BF16, tag="rcb")
        rb = sps.tile([64, NT], BF16, tag="rb")
        with nc.allow_low_precision(reason="bf16 softmax recip for broadcast"):
            nc.vector.reciprocal(rcb[:], acc[64:66, :])
        nc.gpsimd.dma_start(
            rb[:], rcb[:].unsqueeze(1).broadcast_to((2, 32, NT)))
        nc.vector.tensor_tensor(pre[b:b + 64, ci, 0:NT], acc[0:64, :],
                                rb[:], op=ALU.mult)

    if STAGE < 5:
        return
    # ---- dwc (fp8 DoubleRow pairs) accumulated, descaled, added to pre
    for ci in range(2):
        img = vTp[:, ci, :, :]  # dims [(2312,128),(34,34),(1,34)], offset ci*1156
        for hf in range(2):
            acc = ps_a.tile([128, 512], F32, tag="a")
            accv = acc[:].rearrange("p (h w) -> p h w", h=16)
            for pr, (t0, t1) in enumerate(DWC_PAIRS):
                off0 = 34 * (16 * hf + t0[0]) + t0[1]
                delta = 34 * t1[0] + t1[1] - (34 * t0[0] + t0[1])
                src = _sv(img, off0, [(delta, 2), (34, 16), (1, 32)])
                nc.tensor.matmul(accv[:], w3p[:, ci, pr, :, :], src,
                                 start=(pr == 0), stop=False,
                                 perf_mode=PM.DoubleRow)
            dr_, dc_ = DWC_SINGLE
            nc.tensor.matmul(
                accv[:], w3s[:, ci, :],
                vTp[:, ci, 16 * hf + dr_:16 * hf + dr_ + 16, dc_:dc_ + 32],
                start=False, stop=True)
            nc.vector.scalar_tensor_tensor(
                out=pre[:, ci, 512 * hf:512 * hf + 512],
                in0=acc[:], scalar=1.0 / (W_SCALE * V_SCALE),
                in1=pre[:, ci, 512 * hf:512 * hf + 512],
                op0=ALU.mult, op1=ALU.add)

    if STAGE < 6:
        return
    # ---- proj
    for mi in range(2):
        acc = ps_big.tile([128, NT], F32, tag="big")
        for ki in range(2):
            _mm512(nc, acc, wp[:, ki, 128 * mi:128 * mi + 128],
                   pre[:, ki, 0:NT], ki == 0, ki == 1, NT)
        nc.scalar.activation(outT[:, mi, s, 0:NT], acc[:], AF.Copy)
        accc = ps_a.tile([128, 1], F32, tag="a")
        for ki in range(2):
            nc.tensor.matmul(accc[:], wp[:, ki, 128 * mi:128 * mi + 128],
                             pre[:, ki, NT:NT + 1], start=(ki == 0), stop=(ki == 1))
        nc.scalar.activation(outT[:, mi, s, NT:NT + 1], accc[:], AF.Copy)


# ---------------------------------------------------------------- execution
_CACHE = {}


def _get_nc():
    if "nc" not in _CACHE:
        _CACHE["nc"] = build_nc()
    return _CACHE["nc"]


def make_in_maps(x, consts):
    in_maps = []
    for c in range(N_CORES):
        xs = x[SPB * c:SPB * (c + 1)]  # (4, 1025, 256)
        xT = np.ascontiguousarray(
            xs[:, 1:, :].transpose(2, 0, 1)).reshape(2, 128, SPB, NT)
        xcls = np.ascontiguousarray(
            xs[:, 0, :].T).reshape(2, 128, SPB)
        in_maps.append({
            "xT": xT.astype(ml_dtypes.float8_e4m3),
            "xcls": xcls.astype(ml_dtypes.bfloat16),
            "wqk": consts["wqk"], "wv": consts["wv"], "wp": consts["wp"],
            "expB1": np.ascontiguousarray(consts["expB1"]),
            "expB2": np.ascontiguousarray(consts["expB2"]),
            "sa": consts["sa"],
            "w3p": np.ascontiguousarray(consts["w3p"]),
            "w3s": np.ascontiguousarray(consts["w3s"]),
            "ones2": consts["ones2"], "onev": consts["onev"],
        })
    return in_maps


def assemble(results, consts):
    out = np.empty((B, NT + 1, C), np.float32)
    for c in range(N_CORES):
        y = results[c]["y"].astype(np.float32).reshape(C, SPB, NT + 1)
        yT = y.transpose(1, 2, 0)  # (s, t, c)
        out[SPB * c:SPB * (c + 1), 0] = yT[:, NT] + consts["bias_cls"]
        out[SPB * c:SPB * (c + 1), 1:] = yT[:, :NT] + consts["bias_sp"]
    return out


def kernel(x, qkv_w, proj_w, proj_b, dwc_w, dwc_b,
           an_bias, ah_bias, aw_bias, na_bias, ha_bias, wa_bias):
    x = np.asarray(x, np.float32)
    consts = _host_consts(np.asarray(qkv_w, np.float32), np.asarray(proj_w, np.float32),
                          np.asarray(proj_b, np.float32), np.asarray(dwc_w, np.float32),
                          np.asarray(dwc_b, np.float32), np.asarray(an_bias, np.float32),
                          np.asarray(ah_bias, np.float32), np.asarray(aw_bias, np.float32),
                          np.asarray(na_bias, np.float32), np.asarray(ha_bias, np.float32),
                          np.asarray(wa_bias, np.float32))
    nc = _get_nc()
    in_maps = make_in_maps(x, consts)
    res = bass_utils.run_bass_kernel_spmd(nc, in_maps,
                                          core_ids=list(range(N_CORES)))
    return assemble(res.results, consts)


# revision 44
# speedup vs baseline: 1.0366x; 1.0366x over previous
"""AgentAttention Trainium2 kernel (v2).

Data-parallel over batch: 32 samples -> 8 cores x 4 samples.
Device layout is channels-major: activations live as (c, t), bf16.

Per-sample pipeline:
  qk^T  = Wqk^T.T @ xs^T              (bf16 matmuls, f32 PSUM)
  v_t   = xs^T.T @ Wv^T               (tokens-major bf16 + ones col for A1V)
  v^T   = Wv^T.T @ xs^T               (fp8, zero-padded 34x34 image for dwc)
  agents^T: strided-window sums of q^T on DVE (adaptive pool), scaled
  S1^T[t,(h,a)] = k^T.T @ blockdiag(agents) -> exp ACT -> *expB1 (DVE)
  A1V: agent_v + denominators via ones column; normalize -> BDagv
  S2[(h,a),t]  = blockdiag(agents).T @ q^T  -> exp ACT -> *expB2 (DVE)
  A2V: lhsT=[ones2|BDagv] -> dens at PSUM rows 0:2, data rows 2:66;
       recip -> SBUF->SBUF partition-broadcast DMA -> cross-partition mult
  dwc: fp8 DoubleRow diagonal matmuls over shifted views of padded v^T
  proj: Wp^T.T @ pre_proj -> bf16 out
Host adds proj/dwc biases and restores (b, n+1, c) order.
"""

import numpy as np
import ml_dtypes

DEBUG = False
STAGE = 99  # truncate pipeline for perf bisect
REPEAT = 0  # >0: wrap sample loop in a hardware For_i for timing
NOBCAST = False

import bass_rust
import concourse.bacc as bacc
import concourse.tile as tile
import concourse.mybir as mybir
from concourse import bass_utils


def _sv(base_ap, extra_off, dims):
    """Arbitrary-strided free-dim view: keep partition dim, replace free dims
    with explicit (stride, count) pairs, shift the element offset."""
    v = base_ap.copy()
    part = tuple(list(v.ap)[0])
    v.ap = bass_rust.VecI64Pair([part] + [tuple(d) for d in dims])
    v.offset = v.offset + extra_off
    return v

N_CORES = 8
B = 32
SPB = B // N_CORES
C = 256
NT = 1024
WIN = 32
HEADS = 8
HD = 32
AGENT = 49
POOL = 7
SCALE = HD ** -0.5

F32 = mybir.dt.float32
BF16 = mybir.dt.bfloat16
FP8 = mybir.dt.float8e4
AF = mybir.ActivationFunctionType
ALU = mybir.AluOpType
AX = mybir.AxisListType
PM = mybir.MatmulPerfMode

W_SCALE = 32.0   # host scales dwc weights by this for fp8 range
V_SCALE = 4.0    # device scales v image by this for fp8 range
W8S = 32.0       # host scales qkv weights by this for fp8 range

BINS_START = [(i * WIN) // POOL for i in range(POOL)]
BINS_END = [-((-(i + 1) * WIN) // POOL) for i in range(POOL)]

# dwc taps by flat offset in the 34-wide padded image: tap (dr,dc) -> 34*dr+dc
# DoubleRow pairs must have a constant offset delta expressible as one AP dim.
DWC_PAIRS = [((0, 0), (0, 1)), ((0, 2), (1, 0)), ((1, 1), (1, 2)),
             ((2, 0), (2, 1))]
DWC_SINGLE = (2, 2)


# ----------------------------------------------------------------- host prep
def _resize_bilinear_7_to_32(b):
    src, dst = 7, 32
    coords = (np.arange(dst) + 0.5) * (src / dst) - 0.5
    i0 = np.floor(coords).astype(np.int64)
    frac = coords - i0
    i0c = np.clip(i0, 0, src - 1)
    i1c = np.clip(i0 + 1, 0, src - 1)

    def along(x, axis):
        a0 = np.take(x, i0c, axis=axis)
        a1 = np.take(x, i1c, axis=axis)
        sh = [1] * x.ndim
        sh[axis] = dst
        f = frac.reshape(sh)
        return a0 * (1.0 - f) + a1 * f

    return along(along(b, -2), -1)


def _host_consts(qkv_w, proj_w, proj_b, dwc_w, dwc_b,
                 an_bias, ah_bias, aw_bias, na_bias, ha_bias, wa_bias):
    c = {}
    c["wqk"] = np.ascontiguousarray(
        qkv_w[:2 * C].T.reshape(2, 128, 2 * C) * W8S).astype(ml_dtypes.float8_e4m3)
    c["wv"] = np.ascontiguousarray(
        qkv_w[2 * C:].T.reshape(2, 128, C) * W8S).astype(ml_dtypes.float8_e4m3)
    c["wp"] = np.ascontiguousarray(
        proj_w.T.reshape(2, 128, C)).astype(ml_dtypes.bfloat16)

    # stage-1 bias, exp'ed, layout (t, 256*g + 64*h'' + a)
    pb1 = _resize_bilinear_7_to_32(an_bias).reshape(HEADS, AGENT, NT)
    pb2 = (ah_bias + aw_bias).reshape(HEADS, AGENT, NT)
    b1 = pb1 + pb2
    eb1 = np.zeros((NT, 512), np.float32)
    for g in range(2):
        for hh in range(4):
            eb1[:, 256 * g + 64 * hh:256 * g + 64 * hh + AGENT] = \
                b1[4 * g + hh].T
    c["expB1"] = np.exp(eb1).reshape(NT // 128, 128, 512).astype(ml_dtypes.bfloat16)

    # stage-2 bias, exp'ed, layout [pair][64*e + a, t]
    ab1 = _resize_bilinear_7_to_32(na_bias).reshape(HEADS, AGENT, NT)
    ha = ha_bias.reshape(HEADS, AGENT, WIN)
    wa = wa_bias.reshape(HEADS, AGENT, WIN)
    b2 = (ab1.reshape(HEADS, AGENT, WIN, WIN)
          + ha[:, :, :, None] + wa[:, :, None, :]).reshape(HEADS, AGENT, NT)
    eb2 = np.zeros((4, 113, NT), np.float32)
    for p in range(4):
        for e in range(2):
            eb2[p, 64 * e:64 * e + AGENT] = np.exp(b2[2 * p + e])
    c["expB2"] = eb2.astype(ml_dtypes.bfloat16)

    # pooled-agent scale (fold pool mean + attention scale)
    sz = np.array([BINS_END[i] - BINS_START[i] for i in range(POOL)], np.float32)
    sa = SCALE / (sz[:, None] * sz[None, :])
    c["sa"] = np.broadcast_to(sa.reshape(1, AGENT), (128, AGENT)).astype(np.float32).copy()

    # dwc diagonal blocks, fp8, paired for DoubleRow, scaled by W_SCALE
    w3 = dwc_w.reshape(C, 3, 3).astype(np.float32) * W_SCALE
    w3p = np.zeros((2, 4, 128, 2, 128), np.float32)
    for ci in range(2):
        for pr, (t0, t1) in enumerate(DWC_PAIRS):
            for kt, (dr, dc) in enumerate((t0, t1)):
                np.fill_diagonal(w3p[ci, pr, :, kt, :],
                                 w3[128 * ci:128 * ci + 128, dr, dc])
    c["w3p"] = w3p.astype(ml_dtypes.float8_e4m3)
    w3s = np.zeros((2, 128, 128), np.float32)
    for ci in range(2):
        np.fill_diagonal(w3s[ci], w3[128 * ci:128 * ci + 128,
                                     DWC_SINGLE[0], DWC_SINGLE[1]])
    c["w3s"] = w3s.astype(ml_dtypes.float8_e4m3)

    # ones: col0 -> rows 0:49 (e0 den), col1 -> rows 64:113 (e1 den)
    o2 = np.zeros((113, 2), np.float32)
    o2[0:AGENT, 0] = 1.0
    o2[64:64 + AGENT, 1] = 1.0
    c["ones2"] = o2.astype(ml_dtypes.bfloat16)
    c["onev"] = np.ones((128, 1), np.float32).astype(ml_dtypes.bfloat16)

    # host-side output biases
    c["bias_cls"] = proj_b.astype(np.float32)
    c["bias_sp"] = (proj_b + proj_w @ dwc_b).astype(np.float32)
    return c


def _mm512(nc, out, lhsT, rhs, start, stop, n):
    for n0 in range(0, n, 512):
        n1 = min(n0 + 512, n)
        nc.tensor.matmul(out[:, n0:n1], lhsT, rhs[:, n0:n1],
                         start=start, stop=stop)


# ------------------------------------------------------------- device build
def build_nc():
    nc = bacc.Bacc("TRN2", target_bir_lowering=False, debug=False,
                   num_devices=N_CORES)
    dr = {}
    dr["xT"] = nc.dram_tensor("xT", (2, 128, SPB, NT), FP8,
                              kind="ExternalInput").ap()
    dr["xcls"] = nc.dram_tensor("xcls", (2, 128, SPB), BF16,
                                kind="ExternalInput").ap()
    dr["wqk"] = nc.dram_tensor("wqk", (2, 128, 512), FP8, kind="ExternalInput").ap()
    dr["wv"] = nc.dram_tensor("wv", (2, 128, 256), FP8, kind="ExternalInput").ap()
    dr["wp"] = nc.dram_tensor("wp", (2, 128, 256), BF16, kind="ExternalInput").ap()
    dr["expB1"] = nc.dram_tensor("expB1", (8, 128, 512), BF16, kind="ExternalInput").ap()
    dr["expB2"] = nc.dram_tensor("expB2", (4, 113, NT), BF16, kind="ExternalInput").ap()
    dr["sa"] = nc.dram_tensor("sa", (128, AGENT), F32, kind="ExternalInput").ap()
    dr["w3p"] = nc.dram_tensor("w3p", (2, 4, 128, 2, 128), FP8, kind="ExternalInput").ap()
    dr["w3s"] = nc.dram_tensor("w3s", (2, 128, 128), FP8, kind="ExternalInput").ap()
    dr["ones2"] = nc.dram_tensor("ones2", (113, 2), BF16, kind="ExternalInput").ap()
    dr["onev"] = nc.dram_tensor("onev", (128, 1), BF16, kind="ExternalInput").ap()
    dr["y"] = nc.dram_tensor("y", (2, 128, SPB, NT + 1), BF16,
                             kind="ExternalOutput").ap()

    with tile.TileContext(nc) as tc:
        _emit(tc, dr)
    nc.compile()
    return nc


def _emit(tc, dr):
    nc = tc.nc
    from contextlib import ExitStack
    with ExitStack() as ctx:
        cpool = ctx.enter_context(tc.tile_pool(name="consts", bufs=1))
        spx = ctx.enter_context(tc.tile_pool(name="spx", bufs=2))
        sp2 = ctx.enter_context(tc.tile_pool(name="sp2", bufs=3))
        sps = ctx.enter_context(tc.tile_pool(name="sps", bufs=3))
        ps_big = ctx.enter_context(
            tc.tile_pool(name="ps_big", bufs=3, space="PSUM"))
        ps_a = ctx.enter_context(tc.tile_pool(name="ps_a", bufs=2, space="PSUM"))

        # ---- constants to SBUF
        wqk = cpool.tile([128, 2, 512], FP8)
        wv = cpool.tile([128, 2, 256], FP8)
        wp = cpool.tile([128, 2, 256], BF16)
        xcls = cpool.tile([128, 2, SPB], BF16)
        nc.sync.dma_start(xcls[:, 0, :], dr["xcls"][0])
        nc.sync.dma_start(xcls[:, 1, :], dr["xcls"][1])
        eB1 = cpool.tile([128, 8, 512], BF16)
        eB2 = cpool.tile([113, 4, NT], BF16)
        sa = cpool.tile([128, AGENT], F32)
        w3p = cpool.tile([128, 2, 4, 2, 128], FP8)
        w3s = cpool.tile([128, 2, 128], FP8)
        ones2 = cpool.tile([113, 2], BF16)
        onev = cpool.tile([128, 1], BF16)
        for ki in range(2):
            nc.sync.dma_start(wqk[:, ki, :], dr["wqk"][ki])
            nc.sync.dma_start(wv[:, ki, :], dr["wv"][ki])
            nc.sync.dma_start(wp[:, ki, :], dr["wp"][ki])
        for ti in range(8):
            nc.sync.dma_start(eB1[:, ti, :], dr["expB1"][ti])
        for p in range(4):
            nc.sync.dma_start(eB2[:, p, :], dr["expB2"][p])
        nc.sync.dma_start(sa[:], dr["sa"][:])
        for ci in range(2):
            for pr in range(4):
                nc.sync.dma_start(w3p[:, ci, pr, :, :], dr["w3p"][ci, pr])
            nc.sync.dma_start(w3s[:, ci, :], dr["w3s"][ci])
        nc.sync.dma_start(ones2[:], dr["ones2"][:])
        nc.sync.dma_start(onev[:], dr["onev"][:])

        # persistent per-sample ring tiles whose static parts are set up once
        vTps, v_ts, BD1s, BD2s, BDagvs = [], [], [], [], []
        for bi in range(3):
            vTp = cpool.tile([128, 2, 34, 34], FP8, tag=f"vTp{bi}")
            v_t = cpool.tile([128, 8, 4, 65], BF16, tag=f"v_t{bi}")
            BD1 = cpool.tile([128, 2, 256], BF16, tag=f"BD1{bi}")
            BD2 = cpool.tile([128, 4, 113], BF16, tag=f"BD2{bi}")
            BDagv = cpool.tile([113, 4, 66], BF16, tag=f"BDagv{bi}")
            nc.vector.memset(vTp[:], 0.0)
            nc.vector.memset(BD1[:], 0.0)
            nc.vector.memset(BD2[:], 0.0)
            nc.vector.memset(BDagv[:], 0.0)
            for ti in range(8):
                for p4 in range(4):
                    nc.gpsimd.tensor_copy(v_t[:, ti, p4, 64:65], onev[:])
            for p4 in range(4):
                nc.gpsimd.tensor_copy(BDagv[:, p4, 64:66], ones2[:])
            vTps.append(vTp); v_ts.append(v_t)
            BD1s.append(BD1); BD2s.append(BD2); BDagvs.append(BDagv)

        def body():
            xT = spx.tile([128, 2, SPB, NT], FP8, tag="xT")
            for ki in range(2):
                nc.sync.dma_start(xT[:, ki, :, :], dr["xT"][ki])
            outT = spx.tile([128, 2, SPB, NT + 1], BF16, tag="outT")
            for s in range(SPB):
                _sample(tc, dr, s, xT, outT, xcls, wqk, wv, wp, eB1, eB2, sa,
                        w3p, w3s, vTps[s % 3], v_ts[s % 3], BD1s[s % 3],
                        BD2s[s % 3], BDagvs[s % 3],
                        sp2, sps, ps_big, ps_a)
            for mi in range(2):
                nc.sync.dma_start(dr["y"][mi], outT[:, mi, :, :])

        if REPEAT > 0:
            with tc.For_i(0, REPEAT, 1):
                body()
        else:
            body()


def _sample(tc, dr, s, xT, outT, xcls, wqk, wv, wp, eB1, eB2, sa,
            w3p, w3s, vTp, v_t, BD1, BD2, BDagv,
            sp2, sps, ps_big, ps_a):
    nc = tc.nc
    xs = xT[:, :, s, :]  # (128, ki, 1024) spatial only (fp8)

    # ---- qk^T: 4 m-chunks, fp8 DoubleRow over the 2 k-chunks
    qk = sp2.tile([128, 4, NT], BF16, tag="qk")
    for mi in range(4):
        acc = ps_big.tile([128, NT], F32, tag="big")
        for n0 in range(0, NT, 512):
            nc.tensor.matmul(acc[:, n0:n0 + 512],
                             wqk[:, :, 128 * mi:128 * mi + 128],
                             xs[:, :, n0:n0 + 512],
                             start=True, stop=True, perf_mode=PM.DoubleRow)
        if mi >= 2:
            nc.scalar.activation(qk[:, mi, :], acc[:], AF.Copy,
                                 scale=1.0 / W8S)
        else:
            nc.vector.tensor_scalar(out=qk[:, mi, :], in0=acc[:],
                                    scalar1=1.0 / W8S, scalar2=None,
                                    op0=ALU.mult)

    # ---- v tokens-major bf16 (ones col persists at col 64), fp8 DR
    for grp in range(2):
        acc = ps_big.tile([128, 4, 256], F32, tag="big")
        for tj in range(4):
            ti = 4 * grp + tj
            nc.tensor.matmul(acc[:, tj, :],
                             xs[:, :, 128 * ti:128 * ti + 128],
                             wv[:], start=True, stop=True,
                             perf_mode=PM.DoubleRow)
        dst = v_t[:, 4 * grp:4 * grp + 4, :, 0:64]
        srcv = acc[:].rearrange("p t (a b) -> p t a b", a=4)
        if grp == 0:
            nc.vector.tensor_scalar(out=dst, in0=srcv, scalar1=1.0 / W8S,
                                    scalar2=None, op0=ALU.mult)
        else:
            nc.scalar.activation(dst, srcv, AF.Copy, scale=1.0 / W8S)

    # ---- v^T into zero-padded (34,34) fp8 image, scaled, fp8 DR
    for ci in range(2):
        acc = ps_big.tile([128, NT], F32, tag="big")
        for n0 in range(0, NT, 512):
            nc.tensor.matmul(acc[:, n0:n0 + 512],
                             wv[:, :, 128 * ci:128 * ci + 128],
                             xs[:, :, n0:n0 + 512],
                             start=True, stop=True, perf_mode=PM.DoubleRow)
        nc.scalar.activation(
            vTp[:, ci, 1:33, 1:33],
            acc[:].rearrange("p (h w) -> p h w", h=32),
            AF.Copy, scale=V_SCALE / W8S)

    if STAGE < 2:
        return
    # ---- adaptive pool of q -> agents (AG), scaled
    RP = sps.tile([128, 2, POOL, WIN], BF16, tag="RP")
    AGf = sps.tile([128, 2, AGENT], BF16, tag="AGf")
    AG = sps.tile([128, 2, AGENT], BF16, tag="AG")
    qv = qk[:, 0:2, :].rearrange("p c (h w) -> p c w h", h=WIN)
    with nc.allow_low_precision(reason="bf16 pool sums, 21-elem windows"):
        for i in range(POOL):
            nc.vector.reduce_sum(RP[:, :, i, :],
                                 qv[:, :, :, BINS_START[i]:BINS_END[i]], axis=AX.X)
        agv = AGf[:].rearrange("p c (i j) -> p c j i", j=POOL)
        for j in range(POOL):
            nc.vector.reduce_sum(agv[:, :, j, :],
                                 RP[:, :, :, BINS_START[j]:BINS_END[j]], axis=AX.X)
    nc.vector.tensor_tensor(AG[:], AGf[:],
                            sa[:].unsqueeze(1).broadcast_to((128, 2, AGENT)),
                            op=ALU.mult)

    # ---- block-diagonal agent tiles
    for g in range(2):
        for hh in range(4):
            nc.gpsimd.tensor_copy(
                BD1[32 * hh:32 * hh + 32, g, 64 * hh:64 * hh + AGENT],
                AG[32 * hh:32 * hh + 32, g, :])
    for p in range(4):
        b = 64 * (p % 2)
        for e in range(2):
            nc.gpsimd.tensor_copy(
                BD2[b + 32 * e:b + 32 * e + 32, p, 64 * e:64 * e + AGENT],
                AG[b + 32 * e:b + 32 * e + 32, p // 2, :])

    # ---- stage 1 scores^T (t, (g,h,a)) + exp + bias factor
    expS1 = sp2.tile([128, 8, 512], BF16, tag="expS1")
    for ti in range(8):
        acc = ps_a.tile([128, 512], F32, tag="a")
        for g in range(2):
            nc.tensor.matmul(acc[:, 256 * g:256 * g + 256],
                             qk[:, 2 + g, 128 * ti:128 * ti + 128],
                             BD1[:, g, :], start=True, stop=True)
        ev = expS1[:, ti, :].rearrange("p (h c) -> p h c", c=64)[:, :, 0:49]
        av = acc[:].rearrange("p (h c) -> p h c", c=64)[:, :, 0:49]
        nc.scalar.activation(ev, av, AF.Exp)
    # ---- stage 2 scores ((e,a), t) + exp + bias factor
    expS2 = sp2.tile([113, 4, NT], BF16, tag="expS2")
    for p in range(4):
        b = 64 * (p % 2)
        acc = ps_big.tile([113, NT], F32, tag="big")
        _mm512(nc, acc, BD2[b:b + 64, p, :],
               qk[b:b + 64, p // 2, :], True, True, NT)
        nc.scalar.activation(expS2[:, p, :], acc[:], AF.Exp)
    for hf in range(2):
        evh = expS1[:, 4 * hf:4 * hf + 4, :].rearrange(
            "p t (h c) -> p t h c", c=64)[:, :, :, 0:49]
        ebh = eB1[:, 4 * hf:4 * hf + 4, :].rearrange(
            "p t (h c) -> p t h c", c=64)[:, :, :, 0:49]
        nc.vector.tensor_tensor(evh, evh, ebh, op=ALU.mult)

    if STAGE < 3:
        return
    # ---- A1V: agent_v + denominators via ones column -> BDagv cols 2:66
    rec = sps.tile([128, 4, 1], F32, tag="rec")
    for p in range(4):
        acc = ps_a.tile([113, 65], F32, tag="a")
        c0 = 256 * (p // 2) + 128 * (p % 2)
        for ti in range(8):
            nc.tensor.matmul(acc[:], expS1[:, ti, c0:c0 + 113],
                             v_t[:, ti, p, :], start=(ti == 0), stop=(ti == 7))
        nc.vector.reciprocal(rec[0:113, p, :], acc[:, 64:65])
        for e in range(2):
            nc.vector.tensor_scalar(
                out=BDagv[64 * e:64 * e + 49, p, 32 * e:32 * e + 32],
                in0=acc[64 * e:64 * e + 49, 32 * e:32 * e + 32],
                scalar1=rec[64 * e:64 * e + 49, p, :],
                scalar2=None, op0=ALU.mult)

    if STAGE < 4:
        return
    for hf in range(2):
        eng = nc.vector if hf == 0 else nc.gpsimd
        eng.tensor_tensor(expS2[:, 2 * hf:2 * hf + 2, :],
                          expS2[:, 2 * hf:2 * hf + 2, :],
                          eB2[:, 2 * hf:2 * hf + 2, :], op=ALU.mult)

    # ---- A2V: data rows 0:64, dens rows 64:66; bcast recip; normalize
    pre = sp2.tile([128, 2, NT + 1], BF16, tag="pre")
    for ci in range(2):
        nc.gpsimd.tensor_copy(pre[:, ci, NT:NT + 1], xcls[:, ci, s:s + 1])
    for p in range(4):
        b = 64 * (p % 2)
        ci = p // 2
        acc = ps_big.tile([128, NT], F32, tag="big")
        _mm512(nc, acc[0:66, :], BDagv[:, p, :],
               expS2[:, p, :], True, True, NT)
        rcb = sps.tile([2, NT], BF16, tag="rcb")
        rb = sps.tile([64, NT], BF16, tag="rb")
        with nc.allow_low_precision(reason="bf16 softmax recip for broadcast"):
            nc.vector.reciprocal(rcb[:], acc[64:66, :])
        nc.gpsimd.dma_start(
            rb[:], rcb[:].unsqueeze(1).broadcast_to((2, 32, NT)))
        nc.vector.tensor_tensor(pre[b:b + 64, ci, 0:NT], acc[0:64, :],
                                rb[:], op=ALU.mult)

    if STAGE < 5:
        return
    # ---- dwc (fp8 DoubleRow pairs) accumulated, descaled, added to pre
    for ci in range(2):
        img = vTp[:, ci, :, :]  # dims [(2312,128),(34,34),(1,34)], offset ci*1156
        for hf in range(2):
            acc = ps_a.tile([128, 512], F32, tag="a")
            accv = acc[:].rearrange("p (h w) -> p h w", h=16)
            for pr, (t0, t1) in enumerate(DWC_PAIRS):
                off0 = 34 * (16 * hf + t0[0]) + t0[1]
                delta = 34 * t1[0] + t1[1] - (34 * t0[0] + t0[1])
                src = _sv(img, off0, [(delta, 2), (34, 16), (1, 32)])
                nc.tensor.matmul(accv[:], w3p[:, ci, pr, :, :], src,
                                 start=(pr == 0), stop=False,
                                 perf_mode=PM.DoubleRow)
            dr_, dc_ = DWC_SINGLE
            nc.tensor.matmul(
                accv[:], w3s[:, ci, :],
                vTp[:, ci, 16 * hf + dr_:16 * hf + dr_ + 16, dc_:dc_ + 32],
                start=False, stop=True)
            nc.vector.scalar_tensor_tensor(
                out=pre[:, ci, 512 * hf:512 * hf + 512],
                in0=acc[:], scalar=1.0 / (W_SCALE * V_SCALE),
                in1=pre[:, ci, 512 * hf:512 * hf + 512],
                op0=ALU.mult, op1=ALU.add)

    if STAGE < 6:
        return
    # ---- proj
    for mi in range(2):
        acc = ps_big.tile([128, NT], F32, tag="big")
        for ki in range(2):
            _mm512(nc, acc, wp[:, ki, 128 * mi:128 * mi + 128],
                   pre[:, ki, 0:NT], ki == 0, ki == 1, NT)
        nc.scalar.activation(outT[:, mi, s, 0:NT], acc[:], AF.Copy)
        accc = ps_a.tile([128, 1], F32, tag="a")
        for ki in range(2):
            nc.tensor.matmul(accc[:], wp[:, ki, 128 * mi:128 * mi + 128],
                             pre[:, ki, NT:NT + 1], start=(ki == 0), stop=(ki == 1))
        nc.scalar.activation(outT[:, mi, s, NT:NT + 1], accc[:], AF.Copy)


# ---------------------------------------------------------------- execution
_CACHE = {}


def _get_nc():
    if "nc" not in _CACHE:
        _CACHE["nc"] = build_nc()
    return _CACHE["nc"]


def make_in_maps(x, consts):
    in_maps = []
    for c in range(N_CORES):
        xs = x[SPB * c:SPB * (c + 1)]  # (4, 1025, 256)
        xT = np.ascontiguousarray(
            xs[:, 1:, :].transpose(2, 0, 1)).reshape(2, 128, SPB, NT)
        xcls = np.ascontiguousarray(
            xs[:, 0, :].T).reshape(2, 128, SPB)
        in_maps.append({
            "xT": xT.astype(ml_dtypes.float8_e4m3),
            "xcls": xcls.astype(ml_dtypes.bfloat16),
            "wqk": consts["wqk"], "wv": consts["wv"], "wp": consts["wp"],
            "expB1": np.ascontiguousarray(consts["expB1"]),
            "expB2": np.ascontiguousarray(consts["expB2"]),
            "sa": consts["sa"],
            "w3p": np.ascontiguousarray(consts["w3p"]),
            "w3s": np.ascontiguousarray(consts["w3s"]),
            "ones2": consts["ones2"], "onev": consts["onev"],
        })
    return in_maps


def assemble(results, consts):
    out = np.empty((B, NT + 1, C), np.float32)
    for c in range(N_CORES):
        y = results[c]["y"].astype(np.float32).reshape(C, SPB, NT + 1)
        yT = y.transpose(1, 2, 0)  # (s, t, c)
        out[SPB * c:SPB * (c + 1), 0] = yT[:, NT] + consts["bias_cls"]
        out[SPB * c:SPB * (c + 1), 1:] = yT[:, :NT] + consts["bias_sp"]
    return out


def kernel(x, qkv_w, proj_w, proj_b, dwc_w, dwc_b,
           an_bias, ah_bias, aw_bias, na_bias, ha_bias, wa_bias):
    x = np.asarray(x, np.float32)
    consts = _host_consts(np.asarray(qkv_w, np.float32), np.asarray(proj_w, np.float32),
                          np.asarray(proj_b, np.float32), np.asarray(dwc_w, np.float32),
                          np.asarray(dwc_b, np.float32), np.asarray(an_bias, np.float32),
                          np.asarray(ah_bias, np.float32), np.asarray(aw_bias, np.float32),
                          np.asarray(na_bias, np.float32), np.asarray(ha_bias, np.float32),
                          np.asarray(wa_bias, np.float32))
    nc = _get_nc()
    in_maps = make_in_maps(x, consts)
    res = bass_utils.run_bass_kernel_spmd(nc, in_maps,
                                          core_ids=list(range(N_CORES)))
    return assemble(res.results, consts)


# revision 47
# speedup vs baseline: 1.3620x; 1.3139x over previous
"""AgentAttention Trainium2 kernel (v2).

Data-parallel over batch: 32 samples -> 8 cores x 4 samples.
Device layout is channels-major: activations live as (c, t), bf16.

Per-sample pipeline:
  qk^T  = Wqk^T.T @ xs^T              (bf16 matmuls, f32 PSUM)
  v_t   = xs^T.T @ Wv^T               (tokens-major bf16 + ones col for A1V)
  v^T   = Wv^T.T @ xs^T               (fp8, zero-padded 34x34 image for dwc)
  agents^T: strided-window sums of q^T on DVE (adaptive pool), scaled
  S1^T[t,(h,a)] = k^T.T @ blockdiag(agents) -> exp ACT -> *expB1 (DVE)
  A1V: agent_v + denominators via ones column; normalize -> BDagv
  S2[(h,a),t]  = blockdiag(agents).T @ q^T  -> exp ACT -> *expB2 (DVE)
  A2V: lhsT=[ones2|BDagv] -> dens at PSUM rows 0:2, data rows 2:66;
       recip -> SBUF->SBUF partition-broadcast DMA -> cross-partition mult
  dwc: fp8 DoubleRow diagonal matmuls over shifted views of padded v^T
  proj: Wp^T.T @ pre_proj -> bf16 out
Host adds proj/dwc biases and restores (b, n+1, c) order.
"""

import numpy as np
import ml_dtypes

DEBUG = False
STAGE = 99  # truncate pipeline for perf bisect
REPEAT = 0  # >0: wrap sample loop in a hardware For_i for timing
NOBCAST = False

import bass_rust
import concourse.bacc as bacc
import concourse.tile as tile
import concourse.mybir as mybir
from concourse import bass_utils


def _sv(base_ap, extra_off, dims):
    """Arbitrary-strided free-dim view: keep partition dim, replace free dims
    with explicit (stride, count) pairs, shift the element offset."""
    v = base_ap.copy()
    part = tuple(list(v.ap)[0])
    v.ap = bass_rust.VecI64Pair([part] + [tuple(d) for d in dims])
    v.offset = v.offset + extra_off
    return v

N_CORES = 8
B = 32
SPB = B // N_CORES
C = 256
NT = 1024
WIN = 32
HEADS = 8
HD = 32
AGENT = 49
POOL = 7
SCALE = HD ** -0.5

F32 = mybir.dt.float32
BF16 = mybir.dt.bfloat16
FP8 = mybir.dt.float8e4
AF = mybir.ActivationFunctionType
ALU = mybir.AluOpType
AX = mybir.AxisListType
PM = mybir.MatmulPerfMode

W_SCALE = 32.0   # host scales dwc weights by this for fp8 range
V_SCALE = 4.0    # device scales v image by this for fp8 range
W8S = 32.0       # host scales qkv weights by this for fp8 range

BINS_START = [(i * WIN) // POOL for i in range(POOL)]
BINS_END = [-((-(i + 1) * WIN) // POOL) for i in range(POOL)]

# dwc taps by flat offset in the 34-wide padded image: tap (dr,dc) -> 34*dr+dc
# DoubleRow pairs must have a constant offset delta expressible as one AP dim.
DWC_PAIRS = [((0, 0), (0, 1)), ((0, 2), (1, 0)), ((1, 1), (1, 2)),
             ((2, 0), (2, 1))]
DWC_SINGLE = (2, 2)


# ----------------------------------------------------------------- host prep
def _resize_bilinear_7_to_32(b):
    src, dst = 7, 32
    coords = (np.arange(dst) + 0.5) * (src / dst) - 0.5
    i0 = np.floor(coords).astype(np.int64)
    frac = coords - i0
    i0c = np.clip(i0, 0, src - 1)
    i1c = np.clip(i0 + 1, 0, src - 1)

    def along(x, axis):
        a0 = np.take(x, i0c, axis=axis)
        a1 = np.take(x, i1c, axis=axis)
        sh = [1] * x.ndim
        sh[axis] = dst
        f = frac.reshape(sh)
        return a0 * (1.0 - f) + a1 * f

    return along(along(b, -2), -1)


def _host_consts(qkv_w, proj_w, proj_b, dwc_w, dwc_b,
                 an_bias, ah_bias, aw_bias, na_bias, ha_bias, wa_bias):
    c = {}
    c["wqk"] = np.ascontiguousarray(
        qkv_w[:2 * C].T.reshape(2, 128, 2 * C) * W8S).astype(ml_dtypes.float8_e4m3)
    c["wv"] = np.ascontiguousarray(
        qkv_w[2 * C:].T.reshape(2, 128, C) * W8S).astype(ml_dtypes.float8_e4m3)
    c["wp"] = np.ascontiguousarray(
        proj_w.T.reshape(2, 128, C)).astype(ml_dtypes.bfloat16)

    # stage-1 bias, exp'ed, layout (t, 256*g + 64*h'' + a)
    pb1 = _resize_bilinear_7_to_32(an_bias).reshape(HEADS, AGENT, NT)
    pb2 = (ah_bias + aw_bias).reshape(HEADS, AGENT, NT)
    b1 = pb1 + pb2
    eb1 = np.zeros((NT, 512), np.float32)
    for g in range(2):
        for hh in range(4):
            eb1[:, 256 * g + 64 * hh:256 * g + 64 * hh + AGENT] = \
                b1[4 * g + hh].T
    c["expB1"] = np.exp(eb1).reshape(NT // 128, 128, 512).astype(ml_dtypes.bfloat16)

    # stage-2 bias, exp'ed, layout [pair][64*e + a, t]
    ab1 = _resize_bilinear_7_to_32(na_bias).reshape(HEADS, AGENT, NT)
    ha = ha_bias.reshape(HEADS, AGENT, WIN)
    wa = wa_bias.reshape(HEADS, AGENT, WIN)
    b2 = (ab1.reshape(HEADS, AGENT, WIN, WIN)
          + ha[:, :, :, None] + wa[:, :, None, :]).reshape(HEADS, AGENT, NT)
    eb2 = np.zeros((4, 113, NT), np.float32)
    for p in range(4):
        for e in range(2):
            eb2[p, 64 * e:64 * e + AGENT] = np.exp(b2[2 * p + e])
    c["expB2"] = eb2.astype(ml_dtypes.bfloat16)

    # pooled-agent scale (fold pool mean + attention scale)
    sz = np.array([BINS_END[i] - BINS_START[i] for i in range(POOL)], np.float32)
    sa = SCALE / (sz[:, None] * sz[None, :])
    c["sa"] = np.broadcast_to(sa.reshape(1, AGENT), (128, AGENT)).astype(np.float32).copy()

    # dwc diagonal blocks, fp8, paired for DoubleRow, scaled by W_SCALE
    w3 = dwc_w.reshape(C, 3, 3).astype(np.float32) * W_SCALE
    w3p = np.zeros((2, 4, 128, 2, 128), np.float32)
    for ci in range(2):
        for pr, (t0, t1) in enumerate(DWC_PAIRS):
            for kt, (dr, dc) in enumerate((t0, t1)):
                np.fill_diagonal(w3p[ci, pr, :, kt, :],
                                 w3[128 * ci:128 * ci + 128, dr, dc])
    c["w3p"] = w3p.astype(ml_dtypes.float8_e4m3)
    w3s = np.zeros((2, 128, 128), np.float32)
    for ci in range(2):
        np.fill_diagonal(w3s[ci], w3[128 * ci:128 * ci + 128,
                                     DWC_SINGLE[0], DWC_SINGLE[1]])
    c["w3s"] = w3s.astype(ml_dtypes.float8_e4m3)

    # ones: col0 -> rows 0:49 (e0 den), col1 -> rows 64:113 (e1 den)
    o2 = np.zeros((113, 2), np.float32)
    o2[0:AGENT, 0] = 1.0
    o2[64:64 + AGENT, 1] = 1.0
    c["ones2"] = o2.astype(ml_dtypes.bfloat16)
    c["onev"] = np.ones((128, 1), np.float32).astype(ml_dtypes.bfloat16)

    # host-side output biases
    c["bias_cls"] = proj_b.astype(np.float32)
    c["bias_sp"] = (proj_b + proj_w @ dwc_b).astype(np.float32)
    return c


def _mm512(nc, out, lhsT, rhs, start, stop, n):
    for n0 in range(0, n, 512):
        n1 = min(n0 + 512, n)
        nc.tensor.matmul(out[:, n0:n1], lhsT, rhs[:, n0:n1],
                         start=start, stop=stop)


# ------------------------------------------------------------- device build
def build_nc():
    nc = bacc.Bacc("TRN2", target_bir_lowering=False, debug=False,
                   num_devices=N_CORES)
    dr = {}
    dr["xT"] = nc.dram_tensor("xT", (2, 128, SPB, NT), FP8,
                              kind="ExternalInput").ap()
    dr["xcls"] = nc.dram_tensor("xcls", (2, 128, SPB), BF16,
                                kind="ExternalInput").ap()
    dr["wqk"] = nc.dram_tensor("wqk", (2, 128, 512), FP8, kind="ExternalInput").ap()
    dr["wv"] = nc.dram_tensor("wv", (2, 128, 256), FP8, kind="ExternalInput").ap()
    dr["wp"] = nc.dram_tensor("wp", (2, 128, 256), BF16, kind="ExternalInput").ap()
    dr["expB1"] = nc.dram_tensor("expB1", (8, 128, 512), BF16, kind="ExternalInput").ap()
    dr["expB2"] = nc.dram_tensor("expB2", (4, 113, NT), BF16, kind="ExternalInput").ap()
    dr["sa"] = nc.dram_tensor("sa", (128, AGENT), F32, kind="ExternalInput").ap()
    dr["w3p"] = nc.dram_tensor("w3p", (2, 4, 128, 2, 128), FP8, kind="ExternalInput").ap()
    dr["w3s"] = nc.dram_tensor("w3s", (2, 128, 128), FP8, kind="ExternalInput").ap()
    dr["ones2"] = nc.dram_tensor("ones2", (113, 2), BF16, kind="ExternalInput").ap()
    dr["onev"] = nc.dram_tensor("onev", (128, 1), BF16, kind="ExternalInput").ap()
    dr["y"] = nc.dram_tensor("y", (2, 128, SPB, NT + 1), BF16,
                             kind="ExternalOutput").ap()

    with tile.TileContext(nc) as tc:
        _emit(tc, dr)
    nc.compile()
    return nc


def _emit(tc, dr):
    nc = tc.nc
    from contextlib import ExitStack
    with ExitStack() as ctx:
        cpool = ctx.enter_context(tc.tile_pool(name="consts", bufs=1))
        spx = ctx.enter_context(tc.tile_pool(name="spx", bufs=2))
        sp2 = ctx.enter_context(tc.tile_pool(name="sp2", bufs=3))
        sps = ctx.enter_context(tc.tile_pool(name="sps", bufs=3))
        ps_big = ctx.enter_context(
            tc.tile_pool(name="ps_big", bufs=3, space="PSUM"))
        ps_a = ctx.enter_context(tc.tile_pool(name="ps_a", bufs=2, space="PSUM"))

        # ---- constants to SBUF
        wqk = cpool.tile([128, 2, 512], FP8)
        wv = cpool.tile([128, 2, 256], FP8)
        wp = cpool.tile([128, 2, 256], BF16)
        xcls = cpool.tile([128, 2, SPB], BF16)
        nc.sync.dma_start(xcls[:, 0, :], dr["xcls"][0])
        nc.sync.dma_start(xcls[:, 1, :], dr["xcls"][1])
        eB1 = cpool.tile([128, 8, 512], BF16)
        eB2 = cpool.tile([113, 4, NT], BF16)
        sa = cpool.tile([128, AGENT], F32)
        w3p = cpool.tile([128, 2, 4, 2, 128], FP8)
        w3s = cpool.tile([128, 2, 128], FP8)
        ones2 = cpool.tile([113, 2], BF16)
        onev = cpool.tile([128, 1], BF16)
        for ki in range(2):
            nc.sync.dma_start(wqk[:, ki, :], dr["wqk"][ki])
            nc.sync.dma_start(wv[:, ki, :], dr["wv"][ki])
            nc.sync.dma_start(wp[:, ki, :], dr["wp"][ki])
        for ti in range(8):
            nc.sync.dma_start(eB1[:, ti, :], dr["expB1"][ti])
        for p in range(4):
            nc.sync.dma_start(eB2[:, p, :], dr["expB2"][p])
        nc.sync.dma_start(sa[:], dr["sa"][:])
        for ci in range(2):
            for pr in range(4):
                nc.sync.dma_start(w3p[:, ci, pr, :, :], dr["w3p"][ci, pr])
            nc.sync.dma_start(w3s[:, ci, :], dr["w3s"][ci])
        nc.sync.dma_start(ones2[:], dr["ones2"][:])
        nc.sync.dma_start(onev[:], dr["onev"][:])

        # persistent per-sample ring tiles whose static parts are set up once
        vTps, v_ts, BD1s, BD2s, BDagvs = [], [], [], [], []
        for bi in range(3):
            vTp = cpool.tile([128, 2, 34, 34], FP8, tag=f"vTp{bi}")
            v_t = cpool.tile([128, 8, 4, 65], BF16, tag=f"v_t{bi}")
            BD1 = cpool.tile([128, 2, 256], BF16, tag=f"BD1{bi}")
            BD2 = cpool.tile([128, 4, 113], BF16, tag=f"BD2{bi}")
            BDagv = cpool.tile([113, 4, 66], BF16, tag=f"BDagv{bi}")
            nc.vector.memset(vTp[:], 0.0)
            nc.vector.memset(BD1[:], 0.0)
            nc.vector.memset(BD2[:], 0.0)
            nc.vector.memset(BDagv[:], 0.0)
            for ti in range(8):
                for p4 in range(4):
                    nc.gpsimd.tensor_copy(v_t[:, ti, p4, 64:65], onev[:])
            for p4 in range(4):
                nc.gpsimd.tensor_copy(BDagv[:, p4, 64:66], ones2[:])
            vTps.append(vTp); v_ts.append(v_t)
            BD1s.append(BD1); BD2s.append(BD2); BDagvs.append(BDagv)

        def body():
            xT = spx.tile([128, 2, SPB, NT], FP8, tag="xT")
            for ki in range(2):
                nc.sync.dma_start(xT[:, ki, :, :], dr["xT"][ki])
            outT = spx.tile([128, 2, SPB, NT + 1], BF16, tag="outT")
            for s in range(SPB):
                _sample(tc, dr, s, xT, outT, xcls, wqk, wv, wp, eB1, eB2, sa,
                        w3p, w3s, vTps[s % 3], v_ts[s % 3], BD1s[s % 3],
                        BD2s[s % 3], BDagvs[s % 3],
                        sp2, sps, ps_big, ps_a)
                for mi in range(2):
                    nc.sync.dma_start(dr["y"][mi][:, s, :],
                                      outT[:, mi, s, :])

        if REPEAT > 0:
            with tc.For_i(0, REPEAT, 1):
                body()
        else:
            body()


def _sample(tc, dr, s, xT, outT, xcls, wqk, wv, wp, eB1, eB2, sa,
            w3p, w3s, vTp, v_t, BD1, BD2, BDagv,
            sp2, sps, ps_big, ps_a):
    nc = tc.nc
    xs = xT[:, :, s, :]  # (128, ki, 1024) spatial only (fp8)

    # ---- qk^T: 4 m-chunks, fp8 DoubleRow over the 2 k-chunks
    qk = sp2.tile([128, 4, NT], BF16, tag="qk")
    for mi in range(4):
        acc = ps_big.tile([128, NT], F32, tag="big")
        for n0 in range(0, NT, 512):
            nc.tensor.matmul(acc[:, n0:n0 + 512],
                             wqk[:, :, 128 * mi:128 * mi + 128],
                             xs[:, :, n0:n0 + 512],
                             start=True, stop=True, perf_mode=PM.DoubleRow)
        if mi >= 2:
            nc.scalar.activation(qk[:, mi, :], acc[:], AF.Copy,
                                 scale=1.0 / W8S)
        else:
            nc.vector.tensor_scalar(out=qk[:, mi, :], in0=acc[:],
                                    scalar1=1.0 / W8S, scalar2=None,
                                    op0=ALU.mult)

    # ---- v tokens-major bf16 (ones col persists at col 64), fp8 DR
    for grp in range(2):
        acc = ps_big.tile([128, 4, 256], F32, tag="big")
        for tj in range(4):
            ti = 4 * grp + tj
            nc.tensor.matmul(acc[:, tj, :],
                             xs[:, :, 128 * ti:128 * ti + 128],
                             wv[:], start=True, stop=True,
                             perf_mode=PM.DoubleRow)
        dst = v_t[:, 4 * grp:4 * grp + 4, :, 0:64]
        srcv = acc[:].rearrange("p t (a b) -> p t a b", a=4)
        if grp == 0:
            nc.vector.tensor_scalar(out=dst, in0=srcv, scalar1=1.0 / W8S,
                                    scalar2=None, op0=ALU.mult)
        else:
            nc.scalar.activation(dst, srcv, AF.Copy, scale=1.0 / W8S)

    # ---- v^T into zero-padded (34,34) fp8 image, scaled, fp8 DR
    for ci in range(2):
        acc = ps_big.tile([128, NT], F32, tag="big")
        for n0 in range(0, NT, 512):
            nc.tensor.matmul(acc[:, n0:n0 + 512],
                             wv[:, :, 128 * ci:128 * ci + 128],
                             xs[:, :, n0:n0 + 512],
                             start=True, stop=True, perf_mode=PM.DoubleRow)
        nc.scalar.activation(
            vTp[:, ci, 1:33, 1:33],
            acc[:].rearrange("p (h w) -> p h w", h=32),
            AF.Copy, scale=V_SCALE / W8S)

    if STAGE < 2:
        return
    # ---- adaptive pool of q -> agents (AG), scaled
    RP = sps.tile([128, 2, POOL, WIN], BF16, tag="RP")
    AGf = sps.tile([128, 2, AGENT], BF16, tag="AGf")
    AG = sps.tile([128, 2, AGENT], BF16, tag="AG")
    qv = qk[:, 0:2, :].rearrange("p c (h w) -> p c w h", h=WIN)
    with nc.allow_low_precision(reason="bf16 pool sums, 21-elem windows"):
        for i in range(POOL):
            nc.vector.reduce_sum(RP[:, :, i, :],
                                 qv[:, :, :, BINS_START[i]:BINS_END[i]], axis=AX.X)
        agv = AGf[:].rearrange("p c (i j) -> p c j i", j=POOL)
        for j in range(POOL):
            nc.vector.reduce_sum(agv[:, :, j, :],
                                 RP[:, :, :, BINS_START[j]:BINS_END[j]], axis=AX.X)
    nc.vector.tensor_tensor(AG[:], AGf[:],
                            sa[:].unsqueeze(1).broadcast_to((128, 2, AGENT)),
                            op=ALU.mult)

    # ---- block-diagonal agent tiles
    for g in range(2):
        for hh in range(4):
            nc.gpsimd.tensor_copy(
                BD1[32 * hh:32 * hh + 32, g, 64 * hh:64 * hh + AGENT],
                AG[32 * hh:32 * hh + 32, g, :])
    for p in range(4):
        b = 64 * (p % 2)
        for e in range(2):
            nc.gpsimd.tensor_copy(
                BD2[b + 32 * e:b + 32 * e + 32, p, 64 * e:64 * e + AGENT],
                AG[b + 32 * e:b + 32 * e + 32, p // 2, :])

    # ---- stage 1 scores^T (t, (g,h,a)) + exp + bias factor
    expS1 = sp2.tile([128, 8, 512], BF16, tag="expS1")
    for ti in range(8):
        acc = ps_a.tile([128, 512], F32, tag="a")
        for g in range(2):
            nc.tensor.matmul(acc[:, 256 * g:256 * g + 256],
                             qk[:, 2 + g, 128 * ti:128 * ti + 128],
                             BD1[:, g, :], start=True, stop=True)
        ev = expS1[:, ti, :].rearrange("p (h c) -> p h c", c=64)[:, :, 0:49]
        av = acc[:].rearrange("p (h c) -> p h c", c=64)[:, :, 0:49]
        nc.scalar.activation(ev, av, AF.Exp)
    # ---- stage 2 scores ((e,a), t) + exp + bias factor
    expS2 = sp2.tile([113, 4, NT], BF16, tag="expS2")
    for p in range(4):
        b = 64 * (p % 2)
        acc = ps_big.tile([113, NT], F32, tag="big")
        _mm512(nc, acc, BD2[b:b + 64, p, :],
               qk[b:b + 64, p // 2, :], True, True, NT)
        nc.scalar.activation(expS2[:, p, :], acc[:], AF.Exp)
    for hf in range(2):
        evh = expS1[:, 4 * hf:4 * hf + 4, :].rearrange(
            "p t (h c) -> p t h c", c=64)[:, :, :, 0:49]
        ebh = eB1[:, 4 * hf:4 * hf + 4, :].rearrange(
            "p t (h c) -> p t h c", c=64)[:, :, :, 0:49]
        nc.vector.tensor_tensor(evh, evh, ebh, op=ALU.mult)

    if STAGE < 3:
        return
    # ---- A1V: agent_v + denominators via ones column -> BDagv cols 2:66
    rec = sps.tile([128, 4, 1], F32, tag="rec")
    for p in range(4):
        acc = ps_a.tile([113, 65], F32, tag="a")
        c0 = 256 * (p // 2) + 128 * (p % 2)
        for ti in range(8):
            nc.tensor.matmul(acc[:], expS1[:, ti, c0:c0 + 113],
                             v_t[:, ti, p, :], start=(ti == 0), stop=(ti == 7))
        nc.vector.reciprocal(rec[0:113, p, :], acc[:, 64:65])
        for e in range(2):
            nc.vector.tensor_scalar(
                out=BDagv[64 * e:64 * e + 49, p, 32 * e:32 * e + 32],
                in0=acc[64 * e:64 * e + 49, 32 * e:32 * e + 32],
                scalar1=rec[64 * e:64 * e + 49, p, :],
                scalar2=None, op0=ALU.mult)

    if STAGE < 4:
        return
    for hf in range(2):
        eng = nc.vector if hf == 0 else nc.gpsimd
        eng.tensor_tensor(expS2[:, 2 * hf:2 * hf + 2, :],
                          expS2[:, 2 * hf:2 * hf + 2, :],
                          eB2[:, 2 * hf:2 * hf + 2, :], op=ALU.mult)

    # ---- A2V: data rows 0:64, dens rows 64:66; bcast recip; normalize
    pre = sp2.tile([128, 2, NT + 1], BF16, tag="pre")
    for ci in range(2):
        nc.gpsimd.tensor_copy(pre[:, ci, NT:NT + 1], xcls[:, ci, s:s + 1])
    for p in range(4):
        b = 64 * (p % 2)
        ci = p // 2
        acc = ps_big.tile([128, NT], F32, tag="big")
        _mm512(nc, acc[0:66, :], BDagv[:, p, :],
               expS2[:, p, :], True, True, NT)
        rcb = sps.tile([2, NT], BF16, tag="rcb")
        rb = sps.tile([64, NT], BF16, tag="rb")
        with nc.allow_low_precision(reason="bf16 softmax recip for broadcast"):
            nc.vector.reciprocal(rcb[:], acc[64:66, :])
        nc.gpsimd.dma_start(
            rb[:], rcb[:].unsqueeze(1).broadcast_to((2, 32, NT)))
        nc.vector.tensor_tensor(pre[b:b + 64, ci, 0:NT], acc[0:64, :],
                                rb[:], op=ALU.mult)

    if STAGE < 5:
        return
    # ---- dwc (fp8 DoubleRow pairs) accumulated, descaled, added to pre
    for ci in range(2):
        img = vTp[:, ci, :, :]  # dims [(2312,128),(34,34),(1,34)], offset ci*1156
        for hf in range(2):
            acc = ps_a.tile([128, 512], F32, tag="a")
            accv = acc[:].rearrange("p (h w) -> p h w", h=16)
            for pr, (t0, t1) in enumerate(DWC_PAIRS):
                off0 = 34 * (16 * hf + t0[0]) + t0[1]
                delta = 34 * t1[0] + t1[1] - (34 * t0[0] + t0[1])
                src = _sv(img, off0, [(delta, 2), (34, 16), (1, 32)])
                nc.tensor.matmul(accv[:], w3p[:, ci, pr, :, :], src,
                                 start=(pr == 0), stop=False,
                                 perf_mode=PM.DoubleRow)
            dr_, dc_ = DWC_SINGLE
            nc.tensor.matmul(
                accv[:], w3s[:, ci, :],
                vTp[:, ci, 16 * hf + dr_:16 * hf + dr_ + 16, dc_:dc_ + 32],
                start=False, stop=True)
            nc.vector.scalar_tensor_tensor(
                out=pre[:, ci, 512 * hf:512 * hf + 512],
                in0=acc[:], scalar=1.0 / (W_SCALE * V_SCALE),
                in1=pre[:, ci, 512 * hf:512 * hf + 512],
                op0=ALU.mult, op1=ALU.add)

    if STAGE < 6:
        return
    # ---- proj
    for mi in range(2):
        acc = ps_big.tile([128, NT], F32, tag="big")
        for ki in range(2):
            _mm512(nc, acc, wp[:, ki, 128 * mi:128 * mi + 128],
                   pre[:, ki, 0:NT], ki == 0, ki == 1, NT)
        nc.scalar.activation(outT[:, mi, s, 0:NT], acc[:], AF.Copy)
        accc = ps_a.tile([128, 1], F32, tag="a")
        for ki in range(2):
            nc.tensor.matmul(accc[:], wp[:, ki, 128 * mi:128 * mi + 128],
                             pre[:, ki, NT:NT + 1], start=(ki == 0), stop=(ki == 1))
        nc.scalar.activation(outT[:, mi, s, NT:NT + 1], accc[:], AF.Copy)


# ---------------------------------------------------------------- execution
_CACHE = {}


def _get_nc():
    if "nc" not in _CACHE:
        _CACHE["nc"] = build_nc()
    return _CACHE["nc"]


def make_in_maps(x, consts):
    in_maps = []
    for c in range(N_CORES):
        xs = x[SPB * c:SPB * (c + 1)]  # (4, 1025, 256)
        xT = np.ascontiguousarray(
            xs[:, 1:, :].transpose(2, 0, 1)).reshape(2, 128, SPB, NT)
        xcls = np.ascontiguousarray(
            xs[:, 0, :].T).reshape(2, 128, SPB)
        in_maps.append({
            "xT": xT.astype(ml_dtypes.float8_e4m3),
            "xcls": xcls.astype(ml_dtypes.bfloat16),
            "wqk": consts["wqk"], "wv": consts["wv"], "wp": consts["wp"],
            "expB1": np.ascontiguousarray(consts["expB1"]),
            "expB2": np.ascontiguousarray(consts["expB2"]),
            "sa": consts["sa"],
            "w3p": np.ascontiguousarray(consts["w3p"]),
            "w3s": np.ascontiguousarray(consts["w3s"]),
            "ones2": consts["ones2"], "onev": consts["onev"],
        })
    return in_maps


def assemble(results, consts):
    out = np.empty((B, NT + 1, C), np.float32)
    for c in range(N_CORES):
        y = results[c]["y"].astype(np.float32).reshape(C, SPB, NT + 1)
        yT = y.transpose(1, 2, 0)  # (s, t, c)
        out[SPB * c:SPB * (c + 1), 0] = yT[:, NT] + consts["bias_cls"]
        out[SPB * c:SPB * (c + 1), 1:] = yT[:, :NT] + consts["bias_sp"]
    return out


def kernel(x, qkv_w, proj_w, proj_b, dwc_w, dwc_b,
           an_bias, ah_bias, aw_bias, na_bias, ha_bias, wa_bias):
    x = np.asarray(x, np.float32)
    consts = _host_consts(np.asarray(qkv_w, np.float32), np.asarray(proj_w, np.float32),
                          np.asarray(proj_b, np.float32), np.asarray(dwc_w, np.float32),
                          np.asarray(dwc_b, np.float32), np.asarray(an_bias, np.float32),
                          np.asarray(ah_bias, np.float32), np.asarray(aw_bias, np.float32),
                          np.asarray(na_bias, np.float32), np.asarray(ha_bias, np.float32),
                          np.asarray(wa_bias, np.float32))
    nc = _get_nc()
    in_maps = make_in_maps(x, consts)
    res = bass_utils.run_bass_kernel_spmd(nc, in_maps,
                                          core_ids=list(range(N_CORES)))
    return assemble(res.results, consts)
